# revision 13
# baseline (speedup 1.0000x reference)
"""Trainium2 Bass kernel for TernaryLinear: y[b,m,n] = sum_k x[b,m,k] * w[k,n].

Shapes: x (4, 2048, 4096) fp32, w (4096, 4096) ternary fp32 -> y (4, 2048, 4096).

Strategy: flatten x to 8192 rows, row-shard across 8 NeuronCores (1024 rows
each), replicate w. Per core: keep x^T resident in SBUF as 32 k-tiles of
[128k x 1024m] fp16 (the stationary matmul operand; fp16 weights get the
fast-weight-load path so the per-matmul weight load hides under the previous
matmul), stream w as [128k x 512n] fp16 tiles (ternary {-1,0,1} is exact in
fp16), accumulate over K into 8 PSUM banks (one per 128-row m-tile) in fp32,
evict PSUM->SBUF alternating between the vector and scalar engines, DMA
results out. No cross-core communication; host concatenates the row shards.
"""

import sys

for _p in ("/opt/trn_rl_repo", "/opt/pypackages"):
    if _p not in sys.path:
        sys.path.append(_p)

import numpy as np

import concourse.bass as bass
import concourse.bacc as bacc
import concourse.mybir as mybir
import concourse.tile as tile
from concourse.bass_utils import run_bass_kernel_spmd

P = 128
NCORES = 8
B, M, K, N = 4, 2048, 4096, 4096
R = B * M            # 8192 rows total
MR = R // NCORES     # 1024 rows per core
KT = K // P          # 32 k-tiles
MT = MR // P         # 8 m-tiles per core
NCH = 512            # moving free dim per matmul (one PSUM bank of fp32)
NCHUNKS = N // NCH   # 8
F32 = mybir.dt.float32
F16 = mybir.dt.float16

_PROGRAM = None


def _build_program():
    nc = bacc.Bacc(
        "TRN2",
        target_bir_lowering=False,
        debug=False,
        num_devices=NCORES,
    )
    xt = nc.dram_tensor("xt", [P, KT, MT, P], F16, kind="ExternalInput").ap()
    w = nc.dram_tensor("w", [NCHUNKS, KT, P, NCH], F16, kind="ExternalInput").ap()
    y = nc.dram_tensor("y", [MT, P, N], F32, kind="ExternalOutput").ap()

    with tile.TileContext(nc) as tc:
        with (
            tc.tile_pool(name="xres", bufs=1) as xpool,
            tc.tile_pool(name="wstream", bufs=10) as wpool,
            tc.tile_pool(name="outstage", bufs=8) as opool,
            tc.tile_pool(name="acc", bufs=8, space="PSUM") as ppool,
        ):
            # x^T resident: one tile per k-tile, [128 kp, MT, 128 m]. The
            # loads are interleaved with the first n-chunk's w stream (same
            # DMA issue queue) so the PE starts after one x slice + one w
            # tile instead of after the whole x preload.
            xtiles = [None] * KT

            for nch in range(NCHUNKS):
                psums = [
                    ppool.tile([P, NCH], F32, tag="acc", name=f"ps{nch}_{mt}")
                    for mt in range(MT)
                ]
                for kt in range(KT):
                    if nch == 0:
                        xtile = xpool.tile(
                            [P, MT, P], F16, tag=f"x{kt}", name=f"x{kt}"
                        )
                        nc.sync.dma_start(out=xtile[:], in_=xt[:, kt])
                        xtiles[kt] = xtile
                    wt = wpool.tile([P, NCH], F16, tag="w", name=f"w{nch}_{kt}")
                    nc.sync.dma_start(out=wt[:], in_=w[nch, kt])
                    for mt in range(MT):
                        nc.tensor.matmul(
                            out=psums[mt][:],
                            lhsT=xtiles[kt][:, mt, :],
                            rhs=wt[:],
                            start=(kt == 0),
                            stop=(kt == KT - 1),
                        )
                for mt in range(MT):
                    ot = opool.tile([P, NCH], F32, tag="o", name=f"o{nch}_{mt}")
                    if mt % 2 == 0:
                        nc.vector.tensor_copy(ot[:], psums[mt][:])
                    else:
                        nc.scalar.copy(ot[:], psums[mt][:])
                    # Alternate output DMAs across two otherwise-idle engine
                    # queues so they don't serialize behind each other (or
                    # the w-input stream) on one DGE ring.
                    dma_eng = nc.gpsimd if mt % 2 == 0 else nc.scalar
                    dma_eng.dma_start(
                        out=y[mt, :, bass.ts(nch, NCH)], in_=ot[:]
                    )
    nc.compile()
    return nc


def _get_program():
    global _PROGRAM
    if _PROGRAM is None:
        _PROGRAM = _build_program()
    return _PROGRAM


def _prepare_in_maps(x: np.ndarray, w: np.ndarray):
    x = np.ascontiguousarray(x, dtype=np.float32)
    w = np.ascontiguousarray(w, dtype=np.float32)
    # x rows -> [core, mt, mp, kt, kp] -> [core, kp, kt, mt, mp], fp16
    xr = x.reshape(NCORES, MT, P, KT, P)
    xt_all = np.ascontiguousarray(
        xr.transpose(0, 4, 3, 1, 2).astype(np.float16)
    )
    # w [kt, kp, nch, nn] -> [nch, kt, kp, nn], fp16 (exact for ternary)
    wr = np.ascontiguousarray(
        w.reshape(KT, P, NCHUNKS, NCH).transpose(2, 0, 1, 3).astype(np.float16)
    )
    return [{"xt": xt_all[c], "w": wr} for c in range(NCORES)]


def _gather_output(results):
    y = np.stack([np.asarray(r["y"]) for r in results])  # [core, MT, P, N]
    return y.reshape(B, M, N)


def run(x: np.ndarray, w: np.ndarray, trace: bool = False):
    """Returns (y, BassKernelResults)."""
    nc = _get_program()
    in_maps = _prepare_in_maps(x, w)
    res = run_bass_kernel_spmd(
        nc, in_maps, core_ids=list(range(NCORES)), trace=trace
    )
    return _gather_output(res.results), res


def kernel(x: np.ndarray, w: np.ndarray) -> np.ndarray:
    y, _ = run(x, w, trace=False)
    return y


# revision 14
# speedup vs baseline: 1.2028x; 1.2028x over previous
"""Trainium2 Bass kernel for TernaryLinear: y[b,m,n] = sum_k x[b,m,k] * w[k,n].

Shapes: x (4, 2048, 4096) fp32, w (4096, 4096) ternary fp32 -> y (4, 2048, 4096).

Strategy: flatten x to 8192 rows, row-shard across 8 NeuronCores (1024 rows
each), replicate w. Per core: keep x^T resident in SBUF as 32 k-tiles of
[128k x 1024m] fp16 (the stationary matmul operand; fp16 weights get the
fast-weight-load path so the per-matmul weight load hides under the previous
matmul), stream w as [128k x 512n] fp16 tiles (ternary {-1,0,1} is exact in
fp16), accumulate over K into 8 PSUM banks (one per 128-row m-tile) in fp32,
evict PSUM->SBUF alternating between the vector and scalar engines, DMA
results out. No cross-core communication; host concatenates the row shards.
"""

import sys

for _p in ("/opt/trn_rl_repo", "/opt/pypackages"):
    if _p not in sys.path:
        sys.path.append(_p)

import numpy as np

import concourse.bass as bass
import concourse.bacc as bacc
import concourse.mybir as mybir
import concourse.tile as tile
from concourse.bass_utils import run_bass_kernel_spmd

P = 128
NCORES = 8
B, M, K, N = 4, 2048, 4096, 4096
R = B * M            # 8192 rows total
MR = R // NCORES     # 1024 rows per core
KT = K // P          # 32 k-tiles
MT = MR // P         # 8 m-tiles per core
NCH = 512            # moving free dim per matmul (one PSUM bank of fp32)
NCHUNKS = N // NCH   # 8
F32 = mybir.dt.float32
F16 = mybir.dt.float16

_PROGRAM = None


def _build_program():
    nc = bacc.Bacc(
        "TRN2",
        target_bir_lowering=False,
        debug=False,
        num_devices=NCORES,
    )
    xt = nc.dram_tensor("xt", [P, KT, MT, P], F16, kind="ExternalInput").ap()
    w = nc.dram_tensor("w", [NCHUNKS, KT, P, NCH], F16, kind="ExternalInput").ap()
    y = nc.dram_tensor("y", [MT, P, N], F32, kind="ExternalOutput").ap()

    with tile.TileContext(nc) as tc:
        with (
            tc.tile_pool(name="xres", bufs=1) as xpool,
            tc.tile_pool(name="wstream", bufs=10) as wpool,
            tc.tile_pool(name="outstage", bufs=8) as opool,
            tc.tile_pool(name="acc", bufs=8, space="PSUM") as ppool,
        ):
            # x^T resident: one tile per k-tile, [128 kp, MT, 128 m]. The
            # loads are interleaved with the first n-chunk's w stream (same
            # DMA issue queue) so the PE starts after one x slice + one w
            # tile instead of after the whole x preload.
            xtiles = [None] * KT

            for nch in range(NCHUNKS):
                psums = [
                    ppool.tile([P, NCH], F32, tag="acc", name=f"ps{nch}_{mt}")
                    for mt in range(MT)
                ]
                for kt in range(KT):
                    if nch == 0:
                        xtile = xpool.tile(
                            [P, MT, P], F16, tag=f"x{kt}", name=f"x{kt}"
                        )
                        nc.sync.dma_start(out=xtile[:], in_=xt[:, kt])
                        xtiles[kt] = xtile
                    wt = wpool.tile([P, NCH], F16, tag="w", name=f"w{nch}_{kt}")
                    nc.sync.dma_start(out=wt[:], in_=w[nch, kt])
                    for mt in range(MT):
                        nc.tensor.matmul(
                            out=psums[mt][:],
                            lhsT=xtiles[kt][:, mt, :],
                            rhs=wt[:],
                            start=(kt == 0),
                            stop=(kt == KT - 1),
                        )
                for mt in range(MT):
                    ot = opool.tile([P, NCH], F32, tag="o", name=f"o{nch}_{mt}")
                    if mt % 2 == 0:
                        nc.vector.tensor_copy(ot[:], psums[mt][:])
                    else:
                        nc.scalar.copy(ot[:], psums[mt][:])
                    # Alternate output DMAs across two otherwise-idle engine
                    # queues so they don't serialize behind each other (or
                    # the w-input stream) on one DGE ring.
                    dma_eng = nc.scalar if mt % 2 == 0 else nc.sync
                    dma_eng.dma_start(
                        out=y[mt, :, bass.ts(nch, NCH)], in_=ot[:]
                    )
    nc.compile()
    return nc


def _get_program():
    global _PROGRAM
    if _PROGRAM is None:
        _PROGRAM = _build_program()
    return _PROGRAM


def _prepare_in_maps(x: np.ndarray, w: np.ndarray):
    x = np.ascontiguousarray(x, dtype=np.float32)
    w = np.ascontiguousarray(w, dtype=np.float32)
    # x rows -> [core, mt, mp, kt, kp] -> [core, kp, kt, mt, mp], fp16
    xr = x.reshape(NCORES, MT, P, KT, P)
    xt_all = np.ascontiguousarray(
        xr.transpose(0, 4, 3, 1, 2).astype(np.float16)
    )
    # w [kt, kp, nch, nn] -> [nch, kt, kp, nn], fp16 (exact for ternary)
    wr = np.ascontiguousarray(
        w.reshape(KT, P, NCHUNKS, NCH).transpose(2, 0, 1, 3).astype(np.float16)
    )
    return [{"xt": xt_all[c], "w": wr} for c in range(NCORES)]


def _gather_output(results):
    y = np.stack([np.asarray(r["y"]) for r in results])  # [core, MT, P, N]
    return y.reshape(B, M, N)


def run(x: np.ndarray, w: np.ndarray, trace: bool = False):
    """Returns (y, BassKernelResults)."""
    nc = _get_program()
    in_maps = _prepare_in_maps(x, w)
    res = run_bass_kernel_spmd(
        nc, in_maps, core_ids=list(range(NCORES)), trace=trace
    )
    return _gather_output(res.results), res


def kernel(x: np.ndarray, w: np.ndarray) -> np.ndarray:
    y, _ = run(x, w, trace=False)
    return y


# revision 16
# speedup vs baseline: 1.2103x; 1.0062x over previous
"""Trainium2 Bass kernel for TernaryLinear: y[b,m,n] = sum_k x[b,m,k] * w[k,n].

Shapes: x (4, 2048, 4096) fp32, w (4096, 4096) ternary fp32 -> y (4, 2048, 4096).

Strategy: flatten x to 8192 rows, row-shard across 8 NeuronCores (1024 rows
each), replicate w. Per core: keep x^T resident in SBUF as 32 k-tiles of
[128k x 1024m] fp16 (the stationary matmul operand; fp16 weights get the
fast-weight-load path so the per-matmul weight load hides under the previous
matmul), stream w as [128k x 512n] fp16 tiles (ternary {-1,0,1} is exact in
fp16), accumulate over K into 8 PSUM banks (one per 128-row m-tile) in fp32,
evict PSUM->SBUF alternating between the vector and scalar engines, DMA
results out. No cross-core communication; host concatenates the row shards.
"""

import sys

for _p in ("/opt/trn_rl_repo", "/opt/pypackages"):
    if _p not in sys.path:
        sys.path.append(_p)

import numpy as np

import concourse.bass as bass
import concourse.bacc as bacc
import concourse.mybir as mybir
import concourse.tile as tile
from concourse.bass_utils import run_bass_kernel_spmd

P = 128
NCORES = 8
B, M, K, N = 4, 2048, 4096, 4096
R = B * M            # 8192 rows total
MR = R // NCORES     # 1024 rows per core
KT = K // P          # 32 k-tiles
MT = MR // P         # 8 m-tiles per core
NCH = 512            # moving free dim per matmul (one PSUM bank of fp32)
NCHUNKS = N // NCH   # 8
F32 = mybir.dt.float32
F16 = mybir.dt.float16

_PROGRAM = None


def _build_program():
    nc = bacc.Bacc(
        "TRN2",
        target_bir_lowering=False,
        debug=False,
        num_devices=NCORES,
    )
    xt = nc.dram_tensor("xt", [P, KT, MT, P], F16, kind="ExternalInput").ap()
    w = nc.dram_tensor("w", [NCHUNKS, KT, P, NCH], F16, kind="ExternalInput").ap()
    y = nc.dram_tensor("y", [MT, P, N], F32, kind="ExternalOutput").ap()

    with tile.TileContext(nc) as tc:
        with (
            tc.tile_pool(name="xres", bufs=1) as xpool,
            tc.tile_pool(name="wstream", bufs=10) as wpool,
            tc.tile_pool(name="outstage", bufs=8) as opool,
            tc.tile_pool(name="acc", bufs=8, space="PSUM") as ppool,
        ):
            # x^T resident: one tile per k-tile, [128 kp, MT, 128 m]. The
            # loads are interleaved with the first n-chunk's w stream (same
            # DMA issue queue) so the PE starts after one x slice + one w
            # tile instead of after the whole x preload.
            xtiles = [None] * KT

            def evict(nch, mt, ps):
                ot = opool.tile([P, NCH], F32, tag="o", name=f"o{nch}_{mt}")
                if mt % 2 == 0:
                    nc.vector.tensor_copy(ot[:], ps[:])
                else:
                    nc.scalar.copy(ot[:], ps[:])
                # Alternate output DMAs across two HWDGE queues so they don't
                # serialize behind each other (or the w-input stream).
                dma_eng = nc.scalar if mt % 2 == 0 else nc.sync
                dma_eng.dma_start(out=y[mt, :, bass.ts(nch, NCH)], in_=ot[:])

            for nch in range(NCHUNKS - 1):
                psums = [
                    ppool.tile([P, NCH], F32, tag="acc", name=f"ps{nch}_{mt}")
                    for mt in range(MT)
                ]
                for kt in range(KT):
                    if nch == 0:
                        xtile = xpool.tile(
                            [P, MT, P], F16, tag=f"x{kt}", name=f"x{kt}"
                        )
                        nc.sync.dma_start(out=xtile[:], in_=xt[:, kt])
                        xtiles[kt] = xtile
                    wt = wpool.tile([P, NCH], F16, tag="w", name=f"w{nch}_{kt}")
                    # During n-chunk 0 the sync queue is busy with the x
                    # preload; issue w loads on the scalar queue in parallel.
                    (nc.scalar if nch == 0 else nc.sync).dma_start(
                        out=wt[:], in_=w[nch, kt]
                    )
                    for mt in range(MT):
                        nc.tensor.matmul(
                            out=psums[mt][:],
                            lhsT=xtiles[kt][:, mt, :],
                            rhs=wt[:],
                            start=(kt == 0),
                            stop=(kt == KT - 1),
                        )
                for mt in range(MT):
                    evict(nch, mt, psums[mt])

            # Last n-chunk: mt-outer / kt-inner so each m-tile's accumulation
            # finishes early and its eviction + output DMA overlap the
            # remaining matmul stream; only the last m-tile drains after the
            # final matmul. Needs all 32 w tiles live at once (own slots).
            nch = NCHUNKS - 1
            wlast = []
            for kt in range(KT):
                wt = wpool.tile(
                    [P, NCH], F16, tag=f"wl{kt}", name=f"wl{kt}", bufs=1
                )
                nc.sync.dma_start(out=wt[:], in_=w[nch, kt])
                wlast.append(wt)
            for mt in range(MT):
                ps = ppool.tile([P, NCH], F32, tag="acc", name=f"psL_{mt}")
                for kt in range(KT):
                    nc.tensor.matmul(
                        out=ps[:],
                        lhsT=xtiles[kt][:, mt, :],
                        rhs=wlast[kt][:],
                        start=(kt == 0),
                        stop=(kt == KT - 1),
                    )
                evict(nch, mt, ps)
    nc.compile()
    return nc


def _get_program():
    global _PROGRAM
    if _PROGRAM is None:
        _PROGRAM = _build_program()
    return _PROGRAM


def _prepare_in_maps(x: np.ndarray, w: np.ndarray):
    x = np.ascontiguousarray(x, dtype=np.float32)
    w = np.ascontiguousarray(w, dtype=np.float32)
    # x rows -> [core, mt, mp, kt, kp] -> [core, kp, kt, mt, mp], fp16
    xr = x.reshape(NCORES, MT, P, KT, P)
    xt_all = np.ascontiguousarray(
        xr.transpose(0, 4, 3, 1, 2).astype(np.float16)
    )
    # w [kt, kp, nch, nn] -> [nch, kt, kp, nn], fp16 (exact for ternary)
    wr = np.ascontiguousarray(
        w.reshape(KT, P, NCHUNKS, NCH).transpose(2, 0, 1, 3).astype(np.float16)
    )
    return [{"xt": xt_all[c], "w": wr} for c in range(NCORES)]


def _gather_output(results):
    y = np.stack([np.asarray(r["y"]) for r in results])  # [core, MT, P, N]
    return y.reshape(B, M, N)


def run(x: np.ndarray, w: np.ndarray, trace: bool = False):
    """Returns (y, BassKernelResults)."""
    nc = _get_program()
    in_maps = _prepare_in_maps(x, w)
    res = run_bass_kernel_spmd(
        nc, in_maps, core_ids=list(range(NCORES)), trace=trace
    )
    return _gather_output(res.results), res


def kernel(x: np.ndarray, w: np.ndarray) -> np.ndarray:
    y, _ = run(x, w, trace=False)
    return y
